# revision 30
# baseline (speedup 1.0000x reference)
"""DeepTreeLSTM Trainium2 Bass kernel (v7: all-tanh gates, fully
software-pipelined).

B=256 perfect binary trees (511 nodes, BFS layout), ChildSum TreeLSTM
bottom-up + MLP head. Data-parallel over trees: 32 trees per NeuronCore
x 8 cores. Device tensors use a transposed "feature-on-partition"
layout: [H (2 chunks of 128 partitions), columns], columns tree-major.

Key ideas:
- sigma(x) = 0.5*(1+tanh(x/2)): the 0.5 pre-activation scales fold into
  host-prepped weights, the affine into 4x-mode DVE tensor_scalar ops.
  Every gate nonlinearity becomes Tanh, so gate drains merge into few,
  large ACT instructions (the ACT engine is this kernel's roofline:
  ~0.83ns/elem + ~200ns/instr, all dtypes).
- f-gate bias enters PSUM as a K=1 rank-1 matmul (bias row x ones), so
  one bias-free ACT drains both feature chunks.
- iou matmuls fill two [P,3,512] PSUM tiles (3 banks each) that
  ping-pong against the ACT drains; f-gates use the other 2 banks.
- Every block's T=tanh(c), h=o1*T tail is deferred one block (pending
  list) so no engine queue head-of-line blocks on a fresh dependency.
- Phase A (leaves + level 7) runs one super-block stage ahead of the
  level-7 consumer; phase B (levels 6..0) is four independent
  tree-quarter chains statically interleaved into phase A's iterations
  as soon as their level-7 columns exist, then drained in round-robin
  waves. This keeps ACT near 100% for the whole kernel.
- The head's inner-node mean reuses the gpsimd pair sums (h_l+h_r) that
  the level recursion needs anyway, halving the reduce volume.

Contract notes vs the reference: the h input is unused (shape only);
c, b_iou, b_in, b_mid, b_out are all-zero per the problem's input spec,
so the kernel drops them (only U_f_b is a live bias).
"""

import os
import sys

import ml_dtypes
import numpy as np

BFNP = ml_dtypes.bfloat16

for _p in ("/opt/trn_rl_repo", "/root/.axon_site/_ro/trn_rl_repo"):
    if os.path.isdir(_p) and _p not in sys.path:
        sys.path.insert(0, _p)

import concourse.bass as bass
import concourse.mybir as mybir
import concourse.tile as tile
from concourse import bacc
from concourse.bass_utils import run_bass_kernel_spmd

P = 128
F32 = mybir.dt.float32
BF16 = mybir.dt.bfloat16
H = 256           # hidden size (2 partition chunks)
NB = 32           # trees per core
LEAF = 256        # leaves per tree
COLS = NB * LEAF  # leaf columns per core = 8192
BLK = 512
NBLK = COLS // BLK
AF = mybir.ActivationFunctionType
OP = mybir.AluOpType

_PROG = None


def _build_program():
    nc = bacc.Bacc("TRN2", target_bir_lowering=False, debug=False,
                   num_devices=8)

    xT = nc.dram_tensor("xT", [P, 2, COLS], BF16, kind="ExternalInput")
    wiouT = nc.dram_tensor("wiouT", [P, 2, 768], BF16, kind="ExternalInput")
    uiouT = nc.dram_tensor("uiouT", [P, 2, 768], BF16, kind="ExternalInput")
    ufT = nc.dram_tensor("ufT", [P, 2, 256], BF16, kind="ExternalInput")
    ufbr = nc.dram_tensor("ufbr", [1, 2, P], BF16, kind="ExternalInput")
    winT = nc.dram_tensor("winT", [P, 5, P], BF16, kind="ExternalInput")
    emoT = nc.dram_tensor("emoT", [P, NB], BF16, kind="ExternalInput")
    wmidT = nc.dram_tensor("wmidT", [P, 64], F32, kind="ExternalInput")
    woutT = nc.dram_tensor("woutT", [P, 4], F32, kind="ExternalInput")
    out_t = nc.dram_tensor("out_t", [4, NB], F32, kind="ExternalOutput")

    with tile.TileContext(nc) as tc:
        with (
            tc.tile_pool(name="wp", bufs=1) as wp,
            tc.tile_pool(name="pers", bufs=1) as pers,
        ):
            wiou_sb = wp.tile([P, 2, 768], BF16)
            uiou_sb = wp.tile([P, 2, 768], BF16)
            uf_sb = wp.tile([P, 2, 256], BF16)
            ufbr_sb = wp.tile([1, 2, P], BF16)
            ones_sb = wp.tile([1, BLK], BF16)
            win_sb = wp.tile([P, 5, P], BF16)
            emo_sb = wp.tile([P, NB], BF16)
            wmid_sb = wp.tile([P, 64], F32)
            wout_sb = wp.tile([P, 4], F32)
            # split the critical weight loads over more DMA queues
            # (shorter time-to-first-matmul)
            for k in range(2):
                for hh in range(2):
                    cs = slice(hh * 384, hh * 384 + 384)
                    nc.sync.dma_start(wiou_sb[:, k, cs], wiouT[:, k, cs])
                    nc.sync.dma_start(uiou_sb[:, k, cs], uiouT[:, k, cs])
            for sb, dr in ((uf_sb, ufT), (ufbr_sb, ufbr), (win_sb, winT),
                           (emo_sb, emoT), (wmid_sb, wmidT),
                           (wout_sb, woutT)):
                nc.sync.dma_start(sb[:], dr[:])
            nc.vector.memset(ones_sb[:], 1.0)

            h7 = pers.tile([P, 2, NB * 128], BF16)
            c7 = pers.tile([P, 2, NB * 128], BF16)
            hsum = pers.tile([P, 2, NB], F32)
            hlast = pers.tile([P, 2, NB], F32)
            nc.vector.memset(hsum[:], 0.0)

            pending = []

            def flush_pending():
                for fn in pending:
                    fn()
                pending.clear()

            def iou_t6(pps, pool, rhs, w_sb, n, tag):
                """t6 = tanh(W @ rhs): 12 matmuls into two [P,3,n] psum
                tiles (3 banks each, separate tags so consecutive blocks
                ping-pong), each drained by one ACT Tanh."""
                t6 = pool.tile([P, 6, n], BF16, tag="t6", bufs=3,
                               name=f"t6_{tag}")
                for hf in range(2):
                    pg = pps.tile([P, 3, BLK], F32, tag=f"iou{hf}",
                                  name=f"pg_{tag}_{hf}")
                    for g in range(3):
                        mm = hf * 3 + g
                        for k in range(2):
                            nc.tensor.matmul(pg[:, g, :n],
                                             w_sb[:, k, mm * P:(mm + 1) * P],
                                             rhs[:, k, :],
                                             start=(k == 0), stop=(k == 1))
                    nc.scalar.activation(t6[:, 3 * hf:3 * hf + 3, :],
                                         pg[:, :, :n], AF.Tanh)
                return t6

            def cell_mid(pool, t6, out_c, w, d_tag, leaf=False):
                """i*u (+ c_agg already in out_c unless leaf): sigma from
                tanh via tensor_scalar (4x mode), mult/add at 2x. Also
                computes o1=sigma(o) (only needs t6) so the deferred
                tail is just T=tanh(c); h=o1*T."""
                i1 = pool.tile([P, 2, w], BF16, tag="i1", bufs=3,
                               name=f"i1_{d_tag}")
                nc.vector.tensor_scalar(i1[:], t6[:, 0:2, :], 0.5, 0.5,
                                        op0=OP.mult, op1=OP.add)
                if leaf:
                    nc.vector.tensor_mul(out_c, i1[:], t6[:, 4:6, :])
                else:
                    iu = pool.tile([P, 2, w], BF16, tag="iu", bufs=3,
                                   name=f"iu_{d_tag}")
                    nc.vector.tensor_mul(iu[:], i1[:], t6[:, 4:6, :])
                    nc.vector.tensor_add(out_c, iu[:], out_c)
                o1 = pool.tile([P, 2, w], BF16, tag="o1", bufs=4,
                               name=f"o1_{d_tag}")
                nc.vector.tensor_scalar(o1[:], t6[:, 2:4, :], 0.5, 0.5,
                                        op0=OP.mult, op1=OP.add)
                return o1

            def cell_tail(pool, o1, out_h, out_c, w, d_tag):
                """T = tanh(c); h = sigma(o)*T. Deferred one block."""
                ts = pool.tile([P, 2, w], BF16, tag="tb", bufs=3,
                               name=f"t_{d_tag}")
                nc.scalar.activation(ts[:], out_c, AF.Tanh)
                nc.vector.tensor_mul(out_h, o1[:], ts[:])

            def level_body(pool, pps, ch_h, ch_c, out_h, out_c, m, hsum_dst,
                           trees, d_tag):
                """One level chunk: children [P,2,2m] -> parents [P,2,m].

                Per block: f-stage first (reads only stage-old data),
                then flush deferred tails, then pairs/iou/cell so no
                engine queue head-of-line blocks on a fresh dependency.
                """
                fcv = ch_c.rearrange("p k (m two) -> p k m two", two=2)
                hv = ch_h.rearrange("p k (m two) -> p k m two", two=2)
                n_j = (m + BLK - 1) // BLK
                for j in range(n_j):
                    w = min(BLK, m - j * BLK)
                    s = slice(j * BLK, j * BLK + w)
                    # f gates + fc=sigma(f)*c in place over this block's
                    # 2w children
                    for jc in range((2 * w + BLK - 1) // BLK):
                        wc = min(BLK, 2 * w - jc * BLK)
                        sc = slice(2 * j * BLK + jc * BLK,
                                   2 * j * BLK + jc * BLK + wc)
                        pf = pps.tile([P, 2, BLK], F32, tag="f",
                                      name=f"pf_{d_tag}_{j}_{jc}")
                        f_sb = pool.tile([P, 2, wc], BF16, tag="fb", bufs=3,
                                         name=f"f_{d_tag}_{j}_{jc}")
                        for g in range(2):
                            # bias enters PSUM as a K=1 rank-1 matmul so
                            # one bias-free ACT drains both chunks
                            nc.tensor.matmul(pf[:, g, :wc],
                                             ufbr_sb[:, g, :],
                                             ones_sb[:, :wc],
                                             start=True, stop=False)
                            for k in range(2):
                                nc.tensor.matmul(
                                    pf[:, g, :wc],
                                    uf_sb[:, k, g * P:(g + 1) * P],
                                    ch_h[:, k, sc],
                                    start=False, stop=(k == 1))
                        nc.scalar.activation(f_sb[:], pf[:, :, :wc],
                                             AF.Tanh)
                        f1 = pool.tile([P, 2, wc], BF16, tag="f1", bufs=3,
                                       name=f"f1_{d_tag}_{j}_{jc}")
                        nc.vector.tensor_scalar(f1[:], f_sb[:], 0.5, 0.5,
                                                op0=OP.mult, op1=OP.add)
                        nc.vector.tensor_mul(ch_c[:, :, sc], f1[:],
                                             ch_c[:, :, sc])

                    flush_pending()
                    # pair sums (gpsimd, strided reads)
                    ht = pool.tile([P, 2, w], BF16, tag="ht", bufs=3,
                                   name=f"ht_{d_tag}_{j}")
                    nc.gpsimd.tensor_add(out_c[:, :, s], fcv[:, :, s, 0],
                                         fcv[:, :, s, 1])
                    nc.gpsimd.tensor_add(ht[:], hv[:, :, s, 0],
                                         hv[:, :, s, 1])
                    # head's inner mean: summing the pair sums covers the
                    # children level; root is never summed
                    if hsum_dst is not None:
                        tj = trees // n_j
                        hs = slice(j * tj, j * tj + tj)
                        if w // tj > 1:
                            part = pool.tile([P, 2, tj], F32, tag="part",
                                             bufs=2,
                                             name=f"part_{d_tag}_{j}")
                            nc.vector.tensor_reduce(
                                part[:],
                                ht.rearrange("p k (t n) -> p k t n", t=tj),
                                axis=mybir.AxisListType.X, op=OP.add)
                            nc.gpsimd.tensor_add(hsum_dst[:, :, hs],
                                                 part[:],
                                                 hsum_dst[:, :, hs])
                        else:
                            nc.gpsimd.tensor_add(hsum_dst[:, :, hs], ht[:],
                                                 hsum_dst[:, :, hs])
                    t6 = iou_t6(pps, pool, ht[:, :, :w], uiou_sb, w,
                                f"{d_tag}_{j}")
                    o1 = cell_mid(pool, t6, out_c[:, :, s], w,
                                  f"{d_tag}_{j}")
                    pending.append(
                        lambda pool=pool, o1=o1, oh=out_h[:, :, s],
                        oc=out_c[:, :, s], w=w, tg=f"{d_tag}_{j}":
                        cell_tail(pool, o1, oh, oc, w, tg))

            with tc.tile_pool(name="pps", bufs=1, space="PSUM") as pps, \
                    tc.tile_pool(name="pa", bufs=2) as pa:
                # dedicated per-level h/c tiles for phase B: the four
                # tree-quarter chains run far apart in time, so levels
                # cannot share a rotating buffer
                lvh = {7: h7, }
                lvc = {7: c7, }
                for d in range(6, -1, -1):
                    m = NB * (2 ** d)
                    lvh[d] = pa.tile([P, 2, m], BF16, tag=f"h{d}", bufs=1,
                                     name=f"h_{d}")
                    lvc[d] = pa.tile([P, 2, m], BF16, tag=f"c{d}", bufs=1,
                                     name=f"c_{d}")

                def emit_step(d, g):
                    """Phase-B chain step: level d for tree-quarter g."""
                    mg = NB * (2 ** d) // 4
                    sp = slice(g * mg, g * mg + mg)
                    sc = slice(2 * g * mg, 2 * g * mg + 2 * mg)
                    level_body(pa, pps, lvh[d + 1][:, :, sc],
                               lvc[d + 1][:, :, sc], lvh[d][:, :, sp],
                               lvc[d][:, :, sp], mg,
                               hsum[:, :, 8 * g:8 * g + 8], 8, f"B{d}_{g}")

                hls, cls = {}, {}

                def leaf_emit(sb):
                    hl = pa.tile([P, 2, 2 * BLK], BF16, tag="hl",
                                 bufs=2, name=f"hl_{sb}")
                    cl = pa.tile([P, 2, 2 * BLK], BF16, tag="cl",
                                 bufs=2, name=f"cl_{sb}")
                    hls[sb], cls[sb] = hl, cl
                    for half in range(2):
                        b = 2 * sb + half
                        hs = slice(half * BLK, half * BLK + BLK)
                        xk = pa.tile([P, 2, BLK], BF16, tag="xk", bufs=4,
                                     name=f"xk_{b}")
                        for k in range(2):
                            nc.sync.dma_start(
                                xk[:, k], xT[:, k, b * BLK:(b + 1) * BLK])
                        t6 = iou_t6(pps, pa, xk[:], wiou_sb, BLK, f"A{b}")
                        o1 = cell_mid(pa, t6, cl[:, :, hs], BLK,
                                      f"A{b}", leaf=True)
                        pending.append(
                            lambda o1=o1, oh=hl[:, :, hs],
                            oc=cl[:, :, hs], tg=f"A{b}":
                            cell_tail(pa, o1, oh, oc, BLK, tg))
                    # last leaf (tree-local leaf 255) of each tree
                    pending.append(
                        lambda sb=sb, hl=hl:
                        nc.vector.tensor_copy(
                            hlast[:, :, 4 * sb:4 * sb + 4],
                            hl[:, :, 255::256]))

                # phase-B chain steps that can interleave into phase A:
                # quarter g only needs super-blocks 2g, 2g+1
                sched = {3: [(6, 0)], 4: [(5, 0), (6, 1)],
                         5: [(4, 0), (5, 1)], 6: [(3, 0), (4, 1), (6, 2)],
                         7: [(2, 0), (3, 1), (5, 2)]}
                for sb in range(NBLK // 2 + 1):
                    if sb < NBLK // 2:
                        leaf_emit(sb)
                    if sb == 0:
                        flush_pending()  # prologue: no lvl7 call yet to
                        # flush super-block 0's leaf tails
                    if sb >= 1:
                        pv = sb - 1
                        ps = slice(pv * BLK, pv * BLK + BLK)
                        level_body(pa, pps, hls[pv][:], cls[pv][:],
                                   h7[:, :, ps], c7[:, :, ps], BLK,
                                   hsum[:, :, 4 * pv:4 * pv + 4], 4,
                                   f"A{pv}")
                    for d, g in sched.get(sb, ()):
                        emit_step(d, g)

                # drain the remaining chain steps in round-robin waves;
                # the pre-wave flush emits the previous wave's tails so
                # every f-stage reads completed h
                waves = [[(1, 0), (2, 1), (4, 2), (6, 3)],
                         [(0, 0), (1, 1), (3, 2), (5, 3)],
                         [(0, 1), (2, 2), (4, 3)],
                         [(1, 2), (3, 3)],
                         [(0, 2), (2, 3)],
                         [(1, 3)],
                         [(0, 3)]]
                for wave in waves:
                    flush_pending()
                    for d, g in wave:
                        emit_step(d, g)

                # ---- head (fp32 tail; all head biases are zero;
                # the 1/509 inner-mean scale folded into winT) ----
                flush_pending()
                h_root = lvh[0]
                inner = pa.tile([P, 2, NB], BF16)
                nc.vector.tensor_sub(inner[:], hsum[:], hlast[:])
                y2_sb = pa.tile([P, NB], F32)
                nc.vector.memset(y2_sb[:], 0.0)

                pht = pps.tile([P, 2, BLK], F32, tag="f", name="p_head")
                py1 = pht[:, 0, :NB]
                chunks = [h_root[:, 0, :], h_root[:, 1, :],
                          inner[:, 0, :], inner[:, 1, :], emo_sb[:]]
                for k in range(5):
                    nc.tensor.matmul(py1, win_sb[:, k, :], chunks[k],
                                     start=(k == 0), stop=(k == 4))
                y1_sb = pa.tile([P, NB], F32)
                nc.scalar.activation(y1_sb[:], py1, AF.Relu)
                py2 = pht[:64, 1, :NB]
                nc.tensor.matmul(py2, wmid_sb[:], y1_sb[:])
                nc.scalar.activation(y2_sb[:64, :], py2, AF.Relu)
                pht2 = pps.tile([P, 2, BLK], F32, tag="f", name="p_out")
                po = pht2[:4, 0, :NB]
                nc.tensor.matmul(po, wout_sb[:], y2_sb[:])
                o_sb = pa.tile([4, NB], F32)
                nc.scalar.activation(o_sb[:], po, AF.Sigmoid)
                nc.sync.dma_start(out_t[:], o_sb[:])

    nc.finalize()
    return nc


def _chunked(w):
    """[K, M] host array -> [P, K//P, M] device layout (K on partitions)."""
    k, m = w.shape
    return np.ascontiguousarray(w.reshape(k // P, P, m).transpose(1, 0, 2))


def _prep_shared(W_iou, U_iou, b_iou, U_f_w, U_f_b, W_in, b_in, W_mid, b_mid,
                 W_out, b_out):
    f = np.float32
    # sigma(x)=0.5*(1+tanh(x/2)): halve the i,o and f pre-activation
    # weight rows so the device computes tanh(x/2); the 0.5/+0.5 affine
    # happens in 4x-mode DVE tensor_scalar ops. h and c stay true-valued.
    W_iou = np.asarray(W_iou, f).copy()
    W_iou[:512] *= 0.5
    U_iou = np.asarray(U_iou, f).copy()
    U_iou[:512] *= 0.5
    U_f = np.asarray(U_f_w, f) * 0.5
    ufbr_h = np.ascontiguousarray(
        (np.asarray(U_f_b, f) * 0.5).reshape(1, 2, P)).astype(BFNP)
    wiouT = _chunked(np.ascontiguousarray(W_iou.T)).astype(BFNP)
    uiouT = _chunked(np.ascontiguousarray(U_iou.T)).astype(BFNP)
    ufT = _chunked(np.ascontiguousarray(U_f.T)).astype(BFNP)
    # head: inner uses (hsum-hlast) -> fold the 1/509 mean scale.
    W_in = np.asarray(W_in, f).copy()
    W_in[:, 256:512] *= 1.0 / 509.0
    winT = np.zeros((640, P), f)
    winT[:544] = W_in.T
    winT = _chunked(winT).astype(BFNP)
    wmidT = np.ascontiguousarray(np.asarray(W_mid, f).T).astype(f)
    woutT = np.zeros((P, 4), f)
    woutT[:64] = np.asarray(W_out, f).T
    return dict(wiouT=wiouT, uiouT=uiouT, ufT=ufT, ufbr=ufbr_h,
                winT=winT, wmidT=wmidT, woutT=woutT)


def _run(X, emo, shared, trace=False):
    global _PROG
    if _PROG is None:
        _PROG = _build_program()
    nc = _PROG

    in_maps = []
    for cc in range(8):
        Xc = X[cc * NB:(cc + 1) * NB, 255:511, :]
        xT = Xc.transpose(2, 0, 1).reshape(256, COLS)
        xT = np.ascontiguousarray(
            xT.reshape(2, P, COLS).transpose(1, 0, 2)).astype(BFNP)
        emoT = np.zeros((P, NB), BFNP)
        emoT[:32] = emo[cc * NB:(cc + 1) * NB].T.astype(BFNP)
        in_maps.append(dict(xT=xT, emoT=emoT, **shared))

    res = None
    for attempt in range(3):
        try:
            res = run_bass_kernel_spmd(nc, in_maps, core_ids=list(range(8)),
                                       trace=trace)
            break
        except Exception:
            if attempt == 2:
                raise
    out = np.concatenate([res.results[cc]["out_t"].T for cc in range(8)],
                         axis=0)
    return np.ascontiguousarray(out.astype(np.float32)), res


def kernel(X, h, c, emo, W_iou, U_iou, b_iou, U_f_w, U_f_b,
           W_in, b_in, W_mid, b_mid, W_out, b_out, **kwargs):
    X = np.asarray(X, np.float32)
    emo = np.asarray(emo, np.float32)
    shared = _prep_shared(np.asarray(W_iou), np.asarray(U_iou),
                          np.asarray(b_iou), np.asarray(U_f_w),
                          np.asarray(U_f_b), np.asarray(W_in),
                          np.asarray(b_in), np.asarray(W_mid),
                          np.asarray(b_mid), np.asarray(W_out),
                          np.asarray(b_out))
    out, _ = _run(X, emo, shared)
    return out


# revision 32
# speedup vs baseline: 1.0913x; 1.0913x over previous
"""DeepTreeLSTM Trainium2 Bass kernel (v7: all-tanh gates, fully
software-pipelined).

B=256 perfect binary trees (511 nodes, BFS layout), ChildSum TreeLSTM
bottom-up + MLP head. Data-parallel over trees: 32 trees per NeuronCore
x 8 cores. Device tensors use a transposed "feature-on-partition"
layout: [H (2 chunks of 128 partitions), columns], columns tree-major.

Key ideas:
- sigma(x) = 0.5*(1+tanh(x/2)): the 0.5 pre-activation scales fold into
  host-prepped weights, the affine into 4x-mode DVE tensor_scalar ops.
  Every gate nonlinearity becomes Tanh, so gate drains merge into few,
  large ACT instructions (the ACT engine is this kernel's roofline:
  ~0.83ns/elem + ~200ns/instr, all dtypes).
- f-gate bias enters PSUM as a K=1 rank-1 matmul (bias row x ones), so
  one bias-free ACT drains both feature chunks.
- iou matmuls fill two [P,3,512] PSUM tiles (3 banks each) that
  ping-pong against the ACT drains; f-gates use the other 2 banks.
- Every block's T=tanh(c), h=o1*T tail is deferred one block (pending
  list) so no engine queue head-of-line blocks on a fresh dependency.
- Phase A (leaves + level 7) runs one super-block stage ahead of the
  level-7 consumer; phase B (levels 6..0) is four independent
  tree-quarter chains statically interleaved into phase A's iterations
  as soon as their level-7 columns exist, then drained in round-robin
  waves. This keeps ACT near 100% for the whole kernel.
- The head's inner-node mean reuses the gpsimd pair sums (h_l+h_r) that
  the level recursion needs anyway, halving the reduce volume.

Contract notes vs the reference: the h input is unused (shape only);
c, b_iou, b_in, b_mid, b_out are all-zero per the problem's input spec,
so the kernel drops them (only U_f_b is a live bias).
"""

import os
import sys

import ml_dtypes
import numpy as np

BFNP = ml_dtypes.bfloat16

for _p in ("/opt/trn_rl_repo", "/root/.axon_site/_ro/trn_rl_repo"):
    if os.path.isdir(_p) and _p not in sys.path:
        sys.path.insert(0, _p)

import concourse.bass as bass
import concourse.mybir as mybir
import concourse.tile as tile
from concourse import bacc
from concourse.bass_utils import run_bass_kernel_spmd

P = 128
F32 = mybir.dt.float32
BF16 = mybir.dt.bfloat16
H = 256           # hidden size (2 partition chunks)
NB = 32           # trees per core
LEAF = 256        # leaves per tree
COLS = NB * LEAF  # leaf columns per core = 8192
BLK = 512
NBLK = COLS // BLK
AF = mybir.ActivationFunctionType
OP = mybir.AluOpType

_PROG = None


def _build_program():
    nc = bacc.Bacc("TRN2", target_bir_lowering=False, debug=False,
                   num_devices=8)

    xT = nc.dram_tensor("xT", [P, 2, COLS], BF16, kind="ExternalInput")
    wiouT = nc.dram_tensor("wiouT", [P, 2, 768], BF16, kind="ExternalInput")
    uiouT = nc.dram_tensor("uiouT", [P, 2, 768], BF16, kind="ExternalInput")
    ufT = nc.dram_tensor("ufT", [P, 2, 256], BF16, kind="ExternalInput")
    ufbr = nc.dram_tensor("ufbr", [1, 2, P], BF16, kind="ExternalInput")
    winT = nc.dram_tensor("winT", [P, 5, P], BF16, kind="ExternalInput")
    emoT = nc.dram_tensor("emoT", [P, NB], BF16, kind="ExternalInput")
    wmidT = nc.dram_tensor("wmidT", [P, 64], F32, kind="ExternalInput")
    woutT = nc.dram_tensor("woutT", [P, 4], F32, kind="ExternalInput")
    out_t = nc.dram_tensor("out_t", [4, NB], F32, kind="ExternalOutput")

    with tile.TileContext(nc) as tc:
        with (
            tc.tile_pool(name="wp", bufs=1) as wp,
            tc.tile_pool(name="pers", bufs=1) as pers,
        ):
            wiou_sb = wp.tile([P, 2, 768], BF16)
            uiou_sb = wp.tile([P, 2, 768], BF16)
            uf_sb = wp.tile([P, 2, 256], BF16)
            ufbr_sb = wp.tile([1, 2, P], BF16)
            ones_sb = wp.tile([1, BLK], BF16)
            win_sb = wp.tile([P, 5, P], BF16)
            emo_sb = wp.tile([P, NB], BF16)
            wmid_sb = wp.tile([P, 64], F32)
            wout_sb = wp.tile([P, 4], F32)
            # split the critical weight loads over more DMA queues
            # (shorter time-to-first-matmul)
            for k in range(2):
                for hh in range(2):
                    cs = slice(hh * 384, hh * 384 + 384)
                    nc.sync.dma_start(wiou_sb[:, k, cs], wiouT[:, k, cs])
                    nc.sync.dma_start(uiou_sb[:, k, cs], uiouT[:, k, cs])
            for sb, dr in ((uf_sb, ufT), (ufbr_sb, ufbr), (win_sb, winT),
                           (emo_sb, emoT), (wmid_sb, wmidT),
                           (wout_sb, woutT)):
                nc.sync.dma_start(sb[:], dr[:])
            nc.vector.memset(ones_sb[:], 1.0)

            h7 = pers.tile([P, 2, NB * 128], BF16)
            c7 = pers.tile([P, 2, NB * 128], BF16)
            hsum = pers.tile([P, 2, NB], F32)
            hlast = pers.tile([P, 2, NB], F32)
            nc.vector.memset(hsum[:], 0.0)

            pending = []

            def flush_pending():
                for fn in pending:
                    fn()
                pending.clear()

            def iou_t6(pps, pool, rhs, w_sb, n, tag):
                """t6 = tanh(W @ rhs): 12 matmuls into two [P,3,n] psum
                tiles (3 banks each, separate tags so consecutive blocks
                ping-pong), each drained by one ACT Tanh."""
                t6 = pool.tile([P, 6, n], BF16, tag="t6", bufs=3,
                               name=f"t6_{tag}")
                for hf in range(2):
                    pg = pps.tile([P, 3, BLK], F32, tag=f"iou{hf}",
                                  name=f"pg_{tag}_{hf}")
                    for g in range(3):
                        mm = hf * 3 + g
                        for k in range(2):
                            nc.tensor.matmul(pg[:, g, :n],
                                             w_sb[:, k, mm * P:(mm + 1) * P],
                                             rhs[:, k, :],
                                             start=(k == 0), stop=(k == 1))
                    nc.scalar.activation(t6[:, 3 * hf:3 * hf + 3, :],
                                         pg[:, :, :n], AF.Tanh)
                return t6

            def cell_mid(pool, t6, out_c, w, d_tag, leaf=False):
                """i*u (+ c_agg already in out_c unless leaf): sigma from
                tanh via tensor_scalar (4x mode), mult/add at 2x. Also
                computes o1=sigma(o) (only needs t6) so the deferred
                tail is just T=tanh(c); h=o1*T."""
                i1 = pool.tile([P, 2, w], BF16, tag="i1", bufs=3,
                               name=f"i1_{d_tag}")
                nc.vector.tensor_scalar(i1[:], t6[:, 0:2, :], 0.5, 0.5,
                                        op0=OP.mult, op1=OP.add)
                if leaf:
                    nc.vector.tensor_mul(out_c, i1[:], t6[:, 4:6, :])
                else:
                    iu = pool.tile([P, 2, w], BF16, tag="iu", bufs=3,
                                   name=f"iu_{d_tag}")
                    nc.vector.tensor_mul(iu[:], i1[:], t6[:, 4:6, :])
                    nc.vector.tensor_add(out_c, iu[:], out_c)
                o1 = pool.tile([P, 2, w], BF16, tag="o1", bufs=4,
                               name=f"o1_{d_tag}")
                nc.vector.tensor_scalar(o1[:], t6[:, 2:4, :], 0.5, 0.5,
                                        op0=OP.mult, op1=OP.add)
                return o1

            def cell_tail(pool, o1, out_h, out_c, w, d_tag):
                """T = tanh(c); h = sigma(o)*T. Deferred one block."""
                ts = pool.tile([P, 2, w], BF16, tag="tb", bufs=3,
                               name=f"t_{d_tag}")
                nc.scalar.activation(ts[:], out_c, AF.Tanh)
                nc.vector.tensor_mul(out_h, o1[:], ts[:])

            def level_body(pool, pps, ch_h, ch_c, out_h, out_c, m, hsum_dst,
                           trees, d_tag):
                """One level chunk: children [P,2,2m] -> parents [P,2,m].

                Per block: f-stage first (reads only stage-old data),
                then flush deferred tails, then pairs/iou/cell so no
                engine queue head-of-line blocks on a fresh dependency.
                """
                fcv = ch_c.rearrange("p k (m two) -> p k m two", two=2)
                hv = ch_h.rearrange("p k (m two) -> p k m two", two=2)
                n_j = (m + BLK - 1) // BLK
                for j in range(n_j):
                    w = min(BLK, m - j * BLK)
                    s = slice(j * BLK, j * BLK + w)
                    # f gates + fc=sigma(f)*c in place over this block's
                    # 2w children
                    for jc in range((2 * w + BLK - 1) // BLK):
                        wc = min(BLK, 2 * w - jc * BLK)
                        sc = slice(2 * j * BLK + jc * BLK,
                                   2 * j * BLK + jc * BLK + wc)
                        pf = pps.tile([P, 2, BLK], F32, tag="f",
                                      name=f"pf_{d_tag}_{j}_{jc}")
                        f_sb = pool.tile([P, 2, wc], BF16, tag="fb", bufs=3,
                                         name=f"f_{d_tag}_{j}_{jc}")
                        for g in range(2):
                            # bias enters PSUM as a K=1 rank-1 matmul so
                            # one bias-free ACT drains both chunks
                            nc.tensor.matmul(pf[:, g, :wc],
                                             ufbr_sb[:, g, :],
                                             ones_sb[:, :wc],
                                             start=True, stop=False)
                            for k in range(2):
                                nc.tensor.matmul(
                                    pf[:, g, :wc],
                                    uf_sb[:, k, g * P:(g + 1) * P],
                                    ch_h[:, k, sc],
                                    start=False, stop=(k == 1))
                        nc.scalar.activation(f_sb[:], pf[:, :, :wc],
                                             AF.Tanh)
                        f1 = pool.tile([P, 2, wc], BF16, tag="f1", bufs=3,
                                       name=f"f1_{d_tag}_{j}_{jc}")
                        nc.vector.tensor_scalar(f1[:], f_sb[:], 0.5, 0.5,
                                                op0=OP.mult, op1=OP.add)
                        nc.vector.tensor_mul(ch_c[:, :, sc], f1[:],
                                             ch_c[:, :, sc])

                    flush_pending()
                    # pair sums (gpsimd, strided reads)
                    ht = pool.tile([P, 2, w], BF16, tag="ht", bufs=3,
                                   name=f"ht_{d_tag}_{j}")
                    nc.gpsimd.tensor_add(out_c[:, :, s], fcv[:, :, s, 0],
                                         fcv[:, :, s, 1])
                    nc.gpsimd.tensor_add(ht[:], hv[:, :, s, 0],
                                         hv[:, :, s, 1])
                    # head's inner mean: summing the pair sums covers the
                    # children level; root is never summed
                    if hsum_dst is not None:
                        tj = trees // n_j
                        hs = slice(j * tj, j * tj + tj)
                        if w // tj > 1:
                            part = pool.tile([P, 2, tj], F32, tag="part",
                                             bufs=2,
                                             name=f"part_{d_tag}_{j}")
                            nc.vector.tensor_reduce(
                                part[:],
                                ht.rearrange("p k (t n) -> p k t n", t=tj),
                                axis=mybir.AxisListType.X, op=OP.add)
                            nc.gpsimd.tensor_add(hsum_dst[:, :, hs],
                                                 part[:],
                                                 hsum_dst[:, :, hs])
                        else:
                            nc.gpsimd.tensor_add(hsum_dst[:, :, hs], ht[:],
                                                 hsum_dst[:, :, hs])
                    t6 = iou_t6(pps, pool, ht[:, :, :w], uiou_sb, w,
                                f"{d_tag}_{j}")
                    o1 = cell_mid(pool, t6, out_c[:, :, s], w,
                                  f"{d_tag}_{j}")
                    pending.append(
                        lambda pool=pool, o1=o1, oh=out_h[:, :, s],
                        oc=out_c[:, :, s], w=w, tg=f"{d_tag}_{j}":
                        cell_tail(pool, o1, oh, oc, w, tg))

            with tc.tile_pool(name="pps", bufs=1, space="PSUM") as pps, \
                    tc.tile_pool(name="pa", bufs=2) as pa:
                # dedicated per-level h/c tiles for phase B: the four
                # tree-quarter chains run far apart in time, so levels
                # cannot share a rotating buffer
                lvh = {7: h7, }
                lvc = {7: c7, }
                for d in range(6, -1, -1):
                    m = NB * (2 ** d)
                    lvh[d] = pa.tile([P, 2, m], BF16, tag=f"h{d}", bufs=1,
                                     name=f"h_{d}")
                    lvc[d] = pa.tile([P, 2, m], BF16, tag=f"c{d}", bufs=1,
                                     name=f"c_{d}")

                def emit_step(d, g, ng):
                    """Phase-B chain step: level d for tree-group g."""
                    mg = NB * (2 ** d) // ng
                    tg = NB // ng
                    sp = slice(g * mg, g * mg + mg)
                    sc = slice(2 * g * mg, 2 * g * mg + 2 * mg)
                    level_body(pa, pps, lvh[d + 1][:, :, sc],
                               lvc[d + 1][:, :, sc], lvh[d][:, :, sp],
                               lvc[d][:, :, sp], mg,
                               hsum[:, :, tg * g:tg * g + tg], tg,
                               f"B{d}_{g}")

                hls, cls = {}, {}

                def leaf_emit(sb):
                    hl = pa.tile([P, 2, 2 * BLK], BF16, tag="hl",
                                 bufs=2, name=f"hl_{sb}")
                    cl = pa.tile([P, 2, 2 * BLK], BF16, tag="cl",
                                 bufs=2, name=f"cl_{sb}")
                    hls[sb], cls[sb] = hl, cl
                    for half in range(2):
                        b = 2 * sb + half
                        hs = slice(half * BLK, half * BLK + BLK)
                        xk = pa.tile([P, 2, BLK], BF16, tag="xk", bufs=4,
                                     name=f"xk_{b}")
                        for k in range(2):
                            nc.sync.dma_start(
                                xk[:, k], xT[:, k, b * BLK:(b + 1) * BLK])
                        t6 = iou_t6(pps, pa, xk[:], wiou_sb, BLK, f"A{b}")
                        o1 = cell_mid(pa, t6, cl[:, :, hs], BLK,
                                      f"A{b}", leaf=True)
                        pending.append(
                            lambda o1=o1, oh=hl[:, :, hs],
                            oc=cl[:, :, hs], tg=f"A{b}":
                            cell_tail(pa, o1, oh, oc, BLK, tg))
                    # last leaf (tree-local leaf 255) of each tree
                    pending.append(
                        lambda sb=sb, hl=hl:
                        nc.vector.tensor_copy(
                            hlast[:, :, 4 * sb:4 * sb + 4],
                            hl[:, :, 255::256]))

                for sb in range(NBLK // 2 + 1):
                    if sb < NBLK // 2:
                        leaf_emit(sb)
                    if sb == 0:
                        flush_pending()  # prologue: no lvl7 call yet to
                        # flush super-block 0's leaf tails
                    if sb >= 1:
                        pv = sb - 1
                        ps = slice(pv * BLK, pv * BLK + BLK)
                        level_body(pa, pps, hls[pv][:], cls[pv][:],
                                   h7[:, :, ps], c7[:, :, ps], BLK,
                                   hsum[:, :, 4 * pv:4 * pv + 4], 4,
                                   f"A{pv}")

                # phase B: independent tree-group chains leapfrog: group
                # g+1's blocks fill the queues while group g's next
                # level waits on its own tail
                for d in range(6, -1, -1):
                    ng = 4 if d >= 3 else 2
                    for g in range(ng):
                        emit_step(d, g, ng)

                # ---- head (fp32 tail; all head biases are zero;
                # the 1/509 inner-mean scale folded into winT) ----
                flush_pending()
                h_root = lvh[0]
                inner = pa.tile([P, 2, NB], BF16)
                nc.vector.tensor_sub(inner[:], hsum[:], hlast[:])
                y2_sb = pa.tile([P, NB], F32)
                nc.vector.memset(y2_sb[:], 0.0)

                pht = pps.tile([P, 2, BLK], F32, tag="f", name="p_head")
                py1 = pht[:, 0, :NB]
                chunks = [h_root[:, 0, :], h_root[:, 1, :],
                          inner[:, 0, :], inner[:, 1, :], emo_sb[:]]
                for k in range(5):
                    nc.tensor.matmul(py1, win_sb[:, k, :], chunks[k],
                                     start=(k == 0), stop=(k == 4))
                y1_sb = pa.tile([P, NB], F32)
                nc.scalar.activation(y1_sb[:], py1, AF.Relu)
                py2 = pht[:64, 1, :NB]
                nc.tensor.matmul(py2, wmid_sb[:], y1_sb[:])
                nc.scalar.activation(y2_sb[:64, :], py2, AF.Relu)
                pht2 = pps.tile([P, 2, BLK], F32, tag="f", name="p_out")
                po = pht2[:4, 0, :NB]
                nc.tensor.matmul(po, wout_sb[:], y2_sb[:])
                o_sb = pa.tile([4, NB], F32)
                nc.scalar.activation(o_sb[:], po, AF.Sigmoid)
                nc.sync.dma_start(out_t[:], o_sb[:])

    nc.finalize()
    return nc


def _chunked(w):
    """[K, M] host array -> [P, K//P, M] device layout (K on partitions)."""
    k, m = w.shape
    return np.ascontiguousarray(w.reshape(k // P, P, m).transpose(1, 0, 2))


def _prep_shared(W_iou, U_iou, b_iou, U_f_w, U_f_b, W_in, b_in, W_mid, b_mid,
                 W_out, b_out):
    f = np.float32
    # sigma(x)=0.5*(1+tanh(x/2)): halve the i,o and f pre-activation
    # weight rows so the device computes tanh(x/2); the 0.5/+0.5 affine
    # happens in 4x-mode DVE tensor_scalar ops. h and c stay true-valued.
    W_iou = np.asarray(W_iou, f).copy()
    W_iou[:512] *= 0.5
    U_iou = np.asarray(U_iou, f).copy()
    U_iou[:512] *= 0.5
    U_f = np.asarray(U_f_w, f) * 0.5
    ufbr_h = np.ascontiguousarray(
        (np.asarray(U_f_b, f) * 0.5).reshape(1, 2, P)).astype(BFNP)
    wiouT = _chunked(np.ascontiguousarray(W_iou.T)).astype(BFNP)
    uiouT = _chunked(np.ascontiguousarray(U_iou.T)).astype(BFNP)
    ufT = _chunked(np.ascontiguousarray(U_f.T)).astype(BFNP)
    # head: inner uses (hsum-hlast) -> fold the 1/509 mean scale.
    W_in = np.asarray(W_in, f).copy()
    W_in[:, 256:512] *= 1.0 / 509.0
    winT = np.zeros((640, P), f)
    winT[:544] = W_in.T
    winT = _chunked(winT).astype(BFNP)
    wmidT = np.ascontiguousarray(np.asarray(W_mid, f).T).astype(f)
    woutT = np.zeros((P, 4), f)
    woutT[:64] = np.asarray(W_out, f).T
    return dict(wiouT=wiouT, uiouT=uiouT, ufT=ufT, ufbr=ufbr_h,
                winT=winT, wmidT=wmidT, woutT=woutT)


def _run(X, emo, shared, trace=False):
    global _PROG
    if _PROG is None:
        _PROG = _build_program()
    nc = _PROG

    in_maps = []
    for cc in range(8):
        Xc = X[cc * NB:(cc + 1) * NB, 255:511, :]
        xT = Xc.transpose(2, 0, 1).reshape(256, COLS)
        xT = np.ascontiguousarray(
            xT.reshape(2, P, COLS).transpose(1, 0, 2)).astype(BFNP)
        emoT = np.zeros((P, NB), BFNP)
        emoT[:32] = emo[cc * NB:(cc + 1) * NB].T.astype(BFNP)
        in_maps.append(dict(xT=xT, emoT=emoT, **shared))

    res = None
    for attempt in range(3):
        try:
            res = run_bass_kernel_spmd(nc, in_maps, core_ids=list(range(8)),
                                       trace=trace)
            break
        except Exception:
            if attempt == 2:
                raise
    out = np.concatenate([res.results[cc]["out_t"].T for cc in range(8)],
                         axis=0)
    return np.ascontiguousarray(out.astype(np.float32)), res


def kernel(X, h, c, emo, W_iou, U_iou, b_iou, U_f_w, U_f_b,
           W_in, b_in, W_mid, b_mid, W_out, b_out, **kwargs):
    X = np.asarray(X, np.float32)
    emo = np.asarray(emo, np.float32)
    shared = _prep_shared(np.asarray(W_iou), np.asarray(U_iou),
                          np.asarray(b_iou), np.asarray(U_f_w),
                          np.asarray(U_f_b), np.asarray(W_in),
                          np.asarray(b_in), np.asarray(W_mid),
                          np.asarray(b_mid), np.asarray(W_out),
                          np.asarray(b_out))
    out, _ = _run(X, emo, shared)
    return out
